# revision 1
# baseline (speedup 1.0000x reference)
"""Trainium2 Bass kernel for nn_CommNetActor (gnn_message_passing).

Algebraic collapse: every comm layer is linear (no activation), so the whole
post-sigmoid network folds into per-agent decoder matrices on the host:

    out[b] = sum_a sigmoid(O[b,a] @ W_enc + b_enc) @ Z_a + r

with Z_a = Gamma4 @ Wdec_a + E4 @ (sum_a' Wdec_a') and r = c4 @ Wsum + b_dec,
where Gamma4/E4/c4 come from composing the 4 comm layers (the per-batch mean
term is itself linear in s0 = sum_a H0, which is absorbed into Z_a).

Device work per core (batch-sharded, 8192/8 = 1024 batches = 65536 tokens):
  - HWDGE f32 load of O tiles (token-major)
  - PE transpose (f32r, 1.5 cyc/row) to feature-major
  - encoder matmul in f32r (full-rate at free-dim 512), K split across row
    groups so half-A / half-B token quadrants run concurrently
  - ACT sigmoid psum->sbuf with per-partition bias, bf16 output
  - per-agent 64x32 bf16 decoder matmuls accumulating in PSUM
  - PE transpose of [32, batches] result back to token-major, DMA out
"""

import sys
import numpy as np

sys.path.insert(0, "/opt/trn_rl_repo")

import ml_dtypes

BATCH, N_AGENTS, OBS_DIM, D, N_ACT = 8192, 64, 128, 64, 32
N_CORES = 8
B_CORE = BATCH // N_CORES              # 1024 batches per core
TOK_CORE = B_CORE * N_AGENTS           # 65536 tokens per core
NT = 1024                              # tokens per super-tile (16 batches)
N_ST = TOK_CORE // NT                  # 64 super-tiles
SG = 16                                # super-tiles per decoder group
N_G = N_ST // SG                       # 4 groups
GB = SG * (NT // N_AGENTS)             # 256 batches per group
GBH = GB // 2                          # 128 batches per half per group

_CACHE = {}


def _fold_weights(W_enc, b_enc, Ws, bs, W_dec, b_dec):
    """Fold the 4 linear comm layers + decoder into Zdec [64,64,32] and r[32]."""
    A = N_AGENTS
    I = np.eye(D)
    Gamma = I.copy()
    E = np.zeros((D, D))
    c = np.zeros(D)
    Lam = I.copy()
    d = np.zeros(D)
    for W, b in zip(Ws, bs):
        W = W.astype(np.float64)
        b = b.astype(np.float64)
        Wt, Wb = W[:D], W[D:]
        V = Wb / A
        Wp = Wt - V
        U = Wt + (A - 1) * V
        E, c = E @ Wp + Lam @ V, c @ Wp + d @ V + b
        Gamma = Gamma @ Wp
        Lam, d = Lam @ U, d @ U + A * b
    Wd = W_dec.astype(np.float64).reshape(A, D, N_ACT)
    Wsum = Wd.sum(axis=0)
    Zdec = np.einsum("ij,ajk->aik", Gamma, Wd) + (E @ Wsum)[None]
    r = c @ Wsum + b_dec.astype(np.float64)
    return Zdec, r


def _build(loop_reps=1):
    import concourse.bass as bass
    import concourse.bacc as bacc
    import concourse.tile as tile
    from concourse import mybir
    from concourse._compat import get_trn_type

    f32 = mybir.dt.float32
    f32r = mybir.dt.float32r
    bf16 = mybir.dt.bfloat16

    nc = bacc.Bacc(get_trn_type() or "TRN2", target_bir_lowering=False,
                   debug=False, enable_asserts=True, num_devices=N_CORES)

    O_d = nc.dram_tensor("O", [TOK_CORE, OBS_DIM], f32, kind="ExternalInput")
    wenc_d = nc.dram_tensor("Wenc", [OBS_DIM, D], bf16, kind="ExternalInput")
    benc_d = nc.dram_tensor("benc2", [64, 1], f32, kind="ExternalInput")
    zdec_d = nc.dram_tensor("Zdec", [64, N_AGENTS, N_ACT], bf16, kind="ExternalInput")
    r_d = nc.dram_tensor("r2", [32, 1], f32, kind="ExternalInput")
    idf_d = nc.dram_tensor("identf", [64, 64], f32, kind="ExternalInput")
    idb_d = nc.dram_tensor("identb", [128, 128], bf16, kind="ExternalInput")
    out_d = nc.dram_tensor("out", [B_CORE, N_ACT], f32, kind="ExternalOutput")

    O_ap = O_d.ap()
    out_ap = out_d.ap()

    with tile.TileContext(nc) as tc:
        with (
            tc.tile_pool(name="const", bufs=1) as const_pool,
            tc.tile_pool(name="ostage", bufs=3) as ostage_pool,
            tc.tile_pool(name="otsb", bufs=3) as ot_pool,
            tc.tile_pool(name="arena", bufs=2) as arena_pool,
            tc.tile_pool(name="outsb", bufs=2) as outsb_pool,
            tc.tile_pool(name="pt", bufs=2, space="PSUM") as pt_pool,
            tc.tile_pool(name="ph", bufs=2, space="PSUM") as ph_pool,
            tc.tile_pool(name="pd", bufs=1, space="PSUM") as pd_pool,
            tc.tile_pool(name="po", bufs=1, space="PSUM") as po_pool,
        ):
            # constants
            wenc = const_pool.tile([OBS_DIM, D], bf16)
            nc.sync.dma_start(out=wenc[:], in_=wenc_d.ap())
            benc = const_pool.tile([64, 1], f32)
            nc.sync.dma_start(out=benc[:], in_=benc_d.ap())
            zdec = const_pool.tile([64, N_AGENTS, N_ACT], bf16)
            nc.sync.dma_start(out=zdec[:], in_=zdec_d.ap())
            r2 = const_pool.tile([32, 1], f32)
            nc.sync.dma_start(out=r2[:], in_=r_d.ap())
            identf = const_pool.tile([64, 64], f32)
            nc.sync.dma_start(out=identf[:], in_=idf_d.ap())
            identb = const_pool.tile([128, 128], bf16)
            nc.sync.dma_start(out=identb[:], in_=idb_d.ap())

            import contextlib
            loop_cm = (tc.For_i(0, loop_reps, 1) if loop_reps > 1
                       else contextlib.nullcontext())
            with loop_cm:
             for g in range(N_G):
                arena = arena_pool.tile([64, SG * NT], bf16)
                for sl in range(SG):
                    st = g * SG + sl
                    # ---- HWDGE f32 load, then GPSIMD downcast to bf16
                    ostf = ostage_pool.tile([128, 8, OBS_DIM], f32, tag="ostf")
                    nc.sync.dma_start(
                        out=ostf[:],
                        in_=O_ap[st * NT:(st + 1) * NT, :].rearrange(
                            "(th tl) f -> tl th f", tl=128),
                    )
                    ost = ostage_pool.tile([128, 8, OBS_DIM], bf16, tag="ostb")
                    nc.gpsimd.tensor_copy(out=ost[:], in_=ostf[:])
                    # ---- PE transpose (bf16) to feature-major OT [f, token]
                    pt = pt_pool.tile([128, NT], bf16)
                    for th in range(8):
                        nc.tensor.transpose(
                            pt[:, th * 128:(th + 1) * 128],
                            ost[:, th, :], identb[:])
                    ot = ot_pool.tile([128, NT], bf16)
                    nc.vector.tensor_copy(ot[:, 0:512], pt[:, 0:512])
                    nc.scalar.copy(ot[:, 512:1024], pt[:, 512:1024])

                    # ---- encoder: full-K f32r matmuls, one per 512-token half
                    ph = ph_pool.tile([64, NT], f32)
                    for hb in range(2):
                        cs = slice(hb * 512, (hb + 1) * 512)
                        nc.tensor.matmul(
                            ph[:, cs], wenc[:], ot[:, cs],
                            start=True, stop=True)

                    # ---- sigmoid(x + b_enc) -> bf16 arena
                    nc.scalar.activation(
                        out=arena[:, sl * NT:(sl + 1) * NT],
                        in_=ph[:],
                        func=mybir.ActivationFunctionType.Sigmoid,
                        bias=benc[:],
                    )

                # ---- decoder: accumulate over agents in one psum strip
                av = arena[:].rearrange(
                    "p (st b a) -> p st b a", b=16, a=N_AGENTS)
                pd = pd_pool.tile([32, GB], f32)
                for a in range(N_AGENTS):
                    nc.tensor.matmul(
                        pd[:], zdec[:, a, :], av[:, :, :, a],
                        start=(a == 0), stop=(a == N_AGENTS - 1))

                # ---- + r, transpose to token-major, store
                sab = outsb_pool.tile([32, GB], f32, tag="sab")
                nc.scalar.add(sab[:], pd[:], add=r2[:])
                po = po_pool.tile([128, 64], f32)
                for ch in range(2):
                    nc.tensor.transpose(
                        po[:, ch * 32:(ch + 1) * 32],
                        sab[:, ch * 128:(ch + 1) * 128],
                        identf[0:32, 0:32])
                outt = outsb_pool.tile([128, 64], f32, tag="outt")
                nc.vector.tensor_copy(outt[:], po[:])
                for ch in range(2):
                    nc.sync.dma_start(
                        out=out_ap[(g * 2 + ch) * 128:(g * 2 + ch + 1) * 128, :],
                        in_=outt[:, ch * 32:(ch + 1) * 32])

    nc.compile()
    return nc


def _prep_inputs(inputs):
    W_enc = np.asarray(inputs["W_enc"], dtype=np.float32)
    b_enc = np.asarray(inputs["b_enc"], dtype=np.float32)
    Ws = [np.asarray(inputs[f"W{k}"], dtype=np.float32) for k in (1, 2, 3, 4)]
    bs = [np.asarray(inputs[f"b{k}"], dtype=np.float32) for k in (1, 2, 3, 4)]
    W_dec = np.asarray(inputs["W_dec"], dtype=np.float32)
    b_dec = np.asarray(inputs["b_dec"], dtype=np.float32)

    Zdec, r = _fold_weights(W_enc, b_enc, Ws, bs, W_dec, b_dec)
    zdev = np.ascontiguousarray(
        Zdec.transpose(1, 0, 2)).astype(ml_dtypes.bfloat16)  # [64 feat, ag, 32]
    benc2 = b_enc.reshape(64, 1).astype(np.float32)
    r2 = r.reshape(32, 1).astype(np.float32)

    O = np.asarray(inputs["O"], dtype=np.float32)
    common = {
        "Wenc": np.ascontiguousarray(W_enc).astype(ml_dtypes.bfloat16),
        "benc2": benc2,
        "Zdec": zdev,
        "r2": r2,
        "identf": np.eye(64, dtype=np.float32),
        "identb": np.eye(128, dtype=np.float32).astype(ml_dtypes.bfloat16),
    }
    in_maps = []
    for c in range(N_CORES):
        o_shard = np.ascontiguousarray(
            O[c * B_CORE:(c + 1) * B_CORE].reshape(TOK_CORE, OBS_DIM))
        in_maps.append({"O": o_shard, **common})
    return in_maps


def _run(inputs, trace=False):
    from concourse.bass_utils import run_bass_kernel_spmd

    if "nc" not in _CACHE:
        _CACHE["nc"] = _build()
    nc = _CACHE["nc"]
    in_maps = _prep_inputs(inputs)
    res = run_bass_kernel_spmd(nc, in_maps, core_ids=list(range(N_CORES)),
                               trace=trace)
    out = np.concatenate(
        [res.results[c]["out"] for c in range(N_CORES)], axis=0)
    return out.astype(np.float32), res


def kernel(**inputs):
    out, _ = _run(inputs, trace=False)
    return out



# revision 3
# speedup vs baseline: 2.3088x; 2.3088x over previous
"""Trainium2 Bass kernel for nn_CommNetActor (gnn_message_passing).

Algebraic collapse: every comm layer is linear (no activation), so the whole
post-sigmoid network folds into per-agent decoder matrices on the host:

    out[b] = sum_a sigmoid(O[b,a] @ W_enc + b_enc) @ Z_a + r

Device pipeline (batch-sharded, 8192/8 = 1024 batches = 65536 tokens/core):
  - host casts O to bf16 (same numerics as the old on-device GPSIMD cast)
  - HWDGE xbar DMA-transpose loads O feature-major straight from HBM
    (no PE transposes, no GPSIMD cast, no PSUM->SBUF copies)
  - encoder: two col-group-tiled bf16 matmuls put agents a<32 on PSUM
    partitions 0-63 and a>=32 on 64-127
  - ACT sigmoid (+bias) -> bf16 arena [128, batch*pair] layout
  - decoder: 32 K=128 matmuls per group (two agents per matmul) accumulate
    over a PSUM strip [32, batches]
  - +r bias, PE transpose back to batch-major, single batched output store
"""

import sys
import numpy as np

sys.path.insert(0, "/opt/trn_rl_repo")

import ml_dtypes

BATCH, N_AGENTS, OBS_DIM, D, N_ACT = 8192, 64, 128, 64, 32
N_CORES = 8
B_CORE = BATCH // N_CORES              # 1024 batches per core
TOK_CORE = B_CORE * N_AGENTS           # 65536 tokens per core
NT = 1024                              # tokens per super-tile (16 batches)
N_ST = TOK_CORE // NT                  # 64 super-tiles
SG = 32                                # super-tiles per decoder group
N_G = N_ST // SG                       # 2 groups
GB = SG * (NT // N_AGENTS)             # 512 batches per group
DMA_ST = 2                             # super-tiles per input DMA (512 KB)

_CACHE = {}


def _fold_weights(W_enc, b_enc, Ws, bs, W_dec, b_dec):
    """Fold the 4 linear comm layers + decoder into Zdec [64,64,32] and r[32]."""
    A = N_AGENTS
    I = np.eye(D)
    Gamma = I.copy()
    E = np.zeros((D, D))
    c = np.zeros(D)
    Lam = I.copy()
    d = np.zeros(D)
    for W, b in zip(Ws, bs):
        W = W.astype(np.float64)
        b = b.astype(np.float64)
        Wt, Wb = W[:D], W[D:]
        V = Wb / A
        Wp = Wt - V
        U = Wt + (A - 1) * V
        E, c = E @ Wp + Lam @ V, c @ Wp + d @ V + b
        Gamma = Gamma @ Wp
        Lam, d = Lam @ U, d @ U + A * b
    Wd = W_dec.astype(np.float64).reshape(A, D, N_ACT)
    Wsum = Wd.sum(axis=0)
    Zdec = np.einsum("ij,ajk->aik", Gamma, Wd) + (E @ Wsum)[None]
    r = c @ Wsum + b_dec.astype(np.float64)
    return Zdec, r


def _build(loop_reps=1):
    import concourse.bass as bass
    import concourse.bacc as bacc
    import concourse.tile as tile
    from concourse import mybir
    from concourse._compat import get_trn_type

    f32 = mybir.dt.float32
    bf16 = mybir.dt.bfloat16

    nc = bacc.Bacc(get_trn_type() or "TRN2", target_bir_lowering=False,
                   debug=False, enable_asserts=True, num_devices=N_CORES)

    O_d = nc.dram_tensor("Obf", [TOK_CORE, OBS_DIM], bf16, kind="ExternalInput")
    wenc_d = nc.dram_tensor("Wenc", [OBS_DIM, D], bf16, kind="ExternalInput")
    benc_d = nc.dram_tensor("benc128", [128, 1], f32, kind="ExternalInput")
    zpair_d = nc.dram_tensor("Zpair", [128, 32, N_ACT], bf16,
                             kind="ExternalInput")
    r_d = nc.dram_tensor("r2", [32, 1], f32, kind="ExternalInput")
    idf_d = nc.dram_tensor("ident32", [32, 32], f32, kind="ExternalInput")
    out_d = nc.dram_tensor("out", [B_CORE, N_ACT], f32, kind="ExternalOutput")

    O_ap = O_d.ap()
    out_ap = out_d.ap()

    with tile.TileContext(nc) as tc:
        with (
            tc.tile_pool(name="const", bufs=1) as const_pool,
            tc.tile_pool(name="otsb", bufs=3) as ot_pool,
            tc.tile_pool(name="arena", bufs=2) as arena_pool,
            tc.tile_pool(name="outsb", bufs=2) as outsb_pool,
            tc.tile_pool(name="outt", bufs=1) as outt_pool,
            tc.tile_pool(name="ph", bufs=2, space="PSUM") as ph_pool,
            tc.tile_pool(name="pd", bufs=2, space="PSUM") as pd_pool,
            tc.tile_pool(name="po", bufs=1, space="PSUM") as po_pool,
        ):
            # constants
            wenc = const_pool.tile([OBS_DIM, D], bf16)
            nc.sync.dma_start(out=wenc[:], in_=wenc_d.ap())
            benc = const_pool.tile([128, 1], f32)
            nc.sync.dma_start(out=benc[:], in_=benc_d.ap())
            zpair = const_pool.tile([128, 32, N_ACT], bf16)
            nc.sync.dma_start(out=zpair[:], in_=zpair_d.ap())
            r2 = const_pool.tile([32, 1], f32)
            nc.sync.dma_start(out=r2[:], in_=r_d.ap())
            idf = const_pool.tile([32, 32], f32)
            nc.sync.dma_start(out=idf[:], in_=idf_d.ap())

            import contextlib
            loop_cm = (tc.For_i(0, loop_reps, 1) if loop_reps > 1
                       else contextlib.nullcontext())
            with loop_cm:
                outt = outt_pool.tile([128, N_G * 4 * N_ACT], f32)
                for g in range(N_G):
                    arena = arena_pool.tile([128, SG * 512], bf16)
                    ot = None
                    for sl in range(SG):
                        st = g * SG + sl
                        if st % DMA_ST == 0:
                            ot = ot_pool.tile([128, DMA_ST * NT], bf16)
                            nc.sync.dma_start(
                                out=ot[:],
                                in_=O_ap[st * NT:(st + DMA_ST) * NT, :],
                                transpose=True)
                        sub = ot[:, (st % DMA_ST) * NT:(st % DMA_ST + 1) * NT]
                        otr = sub.rearrange("p (b a) -> p b a", a=N_AGENTS)
                        ph = ph_pool.tile([128, 512], f32)
                        nc.tensor.matmul(ph[0:64, :], wenc[:],
                                         otr[:, :, 0:32],
                                         start=True, stop=True,
                                         tile_position=(0, 0))
                        nc.tensor.matmul(ph[64:128, :], wenc[:],
                                         otr[:, :, 32:64],
                                         start=True, stop=True,
                                         tile_position=(0, 64))
                        nc.scalar.activation(
                            out=arena[:, sl * 512:(sl + 1) * 512],
                            in_=ph[:],
                            func=mybir.ActivationFunctionType.Sigmoid,
                            bias=benc[:])

                    # decoder: accumulate agent pairs into one psum strip
                    av = arena[:].rearrange("q (st b a) -> q st b a",
                                            b=16, a=32)
                    pd = pd_pool.tile([32, GB], f32)
                    for p in range(32):
                        nc.tensor.matmul(pd[:], zpair[:, p, :],
                                         av[:, :, :, p],
                                         start=(p == 0), stop=(p == 31))

                    # + r, transpose to batch-major
                    sab = outsb_pool.tile([32, GB], f32)
                    nc.scalar.add(sab[:], pd[:], add=r2[:])
                    po = po_pool.tile([128, 4 * N_ACT], f32)
                    for ch in range(4):
                        nc.tensor.matmul(
                            po[:, ch * N_ACT:(ch + 1) * N_ACT],
                            sab[:, ch * 128:(ch + 1) * 128], idf[:],
                            start=True, stop=True)
                    nc.vector.tensor_copy(
                        outt[:, g * 4 * N_ACT:(g + 1) * 4 * N_ACT], po[:])

                nc.sync.dma_start(
                    out=out_ap.rearrange("(g ch p) c -> p g ch c",
                                         g=N_G, ch=4, p=128),
                    in_=outt[:].rearrange("p (g ch c) -> p g ch c",
                                          g=N_G, ch=4))

    nc.compile()
    return nc


def _prep_inputs(inputs):
    W_enc = np.asarray(inputs["W_enc"], dtype=np.float32)
    b_enc = np.asarray(inputs["b_enc"], dtype=np.float32)
    Ws = [np.asarray(inputs[f"W{k}"], dtype=np.float32) for k in (1, 2, 3, 4)]
    bs = [np.asarray(inputs[f"b{k}"], dtype=np.float32) for k in (1, 2, 3, 4)]
    W_dec = np.asarray(inputs["W_dec"], dtype=np.float32)
    b_dec = np.asarray(inputs["b_dec"], dtype=np.float32)

    Zdec, r = _fold_weights(W_enc, b_enc, Ws, bs, W_dec, b_dec)
    zdev = np.ascontiguousarray(Zdec.transpose(1, 0, 2))  # [64 d, 64 a, 32]
    zpair = np.ascontiguousarray(np.concatenate(
        [zdev[:, 0:32, :], zdev[:, 32:64, :]], axis=0)).astype(
            ml_dtypes.bfloat16)                           # [128, 32, 32]
    benc128 = np.concatenate([b_enc, b_enc]).reshape(128, 1).astype(np.float32)
    r2 = r.reshape(32, 1).astype(np.float32)

    O = np.asarray(inputs["O"], dtype=np.float32)
    Obf = O.astype(ml_dtypes.bfloat16)
    common = {
        "Wenc": np.ascontiguousarray(W_enc).astype(ml_dtypes.bfloat16),
        "benc128": benc128,
        "Zpair": zpair,
        "r2": r2,
        "ident32": np.eye(32, dtype=np.float32),
    }
    in_maps = []
    for c in range(N_CORES):
        o_shard = np.ascontiguousarray(
            Obf[c * B_CORE:(c + 1) * B_CORE].reshape(TOK_CORE, OBS_DIM))
        in_maps.append({"Obf": o_shard, **common})
    return in_maps


def _run(inputs, trace=False):
    from concourse.bass_utils import run_bass_kernel_spmd

    if "nc" not in _CACHE:
        _CACHE["nc"] = _build()
    nc = _CACHE["nc"]
    in_maps = _prep_inputs(inputs)
    res = run_bass_kernel_spmd(nc, in_maps, core_ids=list(range(N_CORES)),
                               trace=trace)
    out = np.concatenate(
        [res.results[c]["out"] for c in range(N_CORES)], axis=0)
    return out.astype(np.float32), res


def kernel(**inputs):
    out, _ = _run(inputs, trace=False)
    return out


# revision 9
# speedup vs baseline: 2.5539x; 1.1061x over previous
"""Trainium2 Bass kernel for nn_CommNetActor (gnn_message_passing).

Algebraic collapse: every comm layer is linear (no activation), so the whole
post-sigmoid network folds into per-agent decoder matrices on the host:

    out[b] = sum_a sigmoid(O[b,a] @ W_enc + b_enc) @ Z_a + r

Device pipeline (batch-sharded, 8192/8 = 1024 batches = 65536 tokens/core):
  - host casts O to bf16 (same numerics as the old on-device GPSIMD cast)
  - HWDGE xbar DMA-transpose loads O feature-major straight from HBM
    (no PE transposes, no GPSIMD cast, no PSUM->SBUF copies)
  - encoder: two col-group-tiled bf16 matmuls put agents a<32 on PSUM
    partitions 0-63 and a>=32 on 64-127
  - ACT sigmoid (+bias) -> bf16 arena [128, batch*pair] layout
  - decoder: 32 K=128 matmuls per group (two agents per matmul) accumulate
    over a PSUM strip [32, batches]
  - +r bias, PE transpose back to batch-major, single batched output store
"""

import sys
import numpy as np

sys.path.insert(0, "/opt/trn_rl_repo")

import ml_dtypes

BATCH, N_AGENTS, OBS_DIM, D, N_ACT = 8192, 64, 128, 64, 32
N_CORES = 8
B_CORE = BATCH // N_CORES              # 1024 batches per core
TOK_CORE = B_CORE * N_AGENTS           # 65536 tokens per core
NT = 1024                              # tokens per super-tile (16 batches)
N_ST = TOK_CORE // NT                  # 64 super-tiles
SG = 32                                # super-tiles per decoder group
N_G = N_ST // SG                       # 2 groups
GB = SG * (NT // N_AGENTS)             # 512 batches per group
DMA_ST = 4                             # super-tiles per input DMA (1 MB)

_CACHE = {}


def _fold_weights(W_enc, b_enc, Ws, bs, W_dec, b_dec):
    """Fold the 4 linear comm layers + decoder into Zdec [64,64,32] and r[32]."""
    A = N_AGENTS
    I = np.eye(D)
    Gamma = I.copy()
    E = np.zeros((D, D))
    c = np.zeros(D)
    Lam = I.copy()
    d = np.zeros(D)
    for W, b in zip(Ws, bs):
        W = W.astype(np.float64)
        b = b.astype(np.float64)
        Wt, Wb = W[:D], W[D:]
        V = Wb / A
        Wp = Wt - V
        U = Wt + (A - 1) * V
        E, c = E @ Wp + Lam @ V, c @ Wp + d @ V + b
        Gamma = Gamma @ Wp
        Lam, d = Lam @ U, d @ U + A * b
    Wd = W_dec.astype(np.float64).reshape(A, D, N_ACT)
    Wsum = Wd.sum(axis=0)
    Zdec = np.einsum("ij,ajk->aik", Gamma, Wd) + (E @ Wsum)[None]
    r = c @ Wsum + b_dec.astype(np.float64)
    return Zdec, r


def _build(loop_reps=1):
    import concourse.bass as bass
    import concourse.bacc as bacc
    import concourse.tile as tile
    from concourse import mybir
    from concourse._compat import get_trn_type

    f32 = mybir.dt.float32
    bf16 = mybir.dt.bfloat16

    nc = bacc.Bacc(get_trn_type() or "TRN2", target_bir_lowering=False,
                   debug=False, enable_asserts=True, num_devices=N_CORES)

    O_d = nc.dram_tensor("Obf", [TOK_CORE, OBS_DIM], bf16, kind="ExternalInput")
    wenc_d = nc.dram_tensor("Wenc", [OBS_DIM, D], bf16, kind="ExternalInput")
    benc_d = nc.dram_tensor("benc128", [128, 1], f32, kind="ExternalInput")
    zpair_d = nc.dram_tensor("Zpair", [128, 32, N_ACT], bf16,
                             kind="ExternalInput")
    r_d = nc.dram_tensor("r2", [32, 1], f32, kind="ExternalInput")
    idf_d = nc.dram_tensor("ident32", [32, 32], f32, kind="ExternalInput")
    out_d = nc.dram_tensor("out", [B_CORE, N_ACT], f32, kind="ExternalOutput")

    O_ap = O_d.ap()
    out_ap = out_d.ap()

    with tile.TileContext(nc) as tc:
        with (
            tc.tile_pool(name="const", bufs=1) as const_pool,
            tc.tile_pool(name="otsb", bufs=4) as ot_pool,
            tc.tile_pool(name="arena", bufs=2) as arena_pool,
            tc.tile_pool(name="outsb", bufs=2) as outsb_pool,
            tc.tile_pool(name="outt", bufs=2) as outt_pool,
            tc.tile_pool(name="ph", bufs=4, space="PSUM") as ph_pool,
            tc.tile_pool(name="pd", bufs=2, space="PSUM") as pd_pool,
            tc.tile_pool(name="po", bufs=1, space="PSUM") as po_pool,
        ):
            # constants
            wenc = const_pool.tile([OBS_DIM, D], bf16)
            nc.sync.dma_start(out=wenc[:], in_=wenc_d.ap())
            benc = const_pool.tile([128, 1], f32)
            nc.sync.dma_start(out=benc[:], in_=benc_d.ap())
            zpair = const_pool.tile([128, 32, N_ACT], bf16)
            nc.sync.dma_start(out=zpair[:], in_=zpair_d.ap())
            r2 = const_pool.tile([32, 1], f32)
            nc.sync.dma_start(out=r2[:], in_=r_d.ap())
            idf = const_pool.tile([32, 32], f32)
            nc.sync.dma_start(out=idf[:], in_=idf_d.ap())

            import contextlib
            loop_cm = (tc.For_i(0, loop_reps, 1) if loop_reps > 1
                       else contextlib.nullcontext())
            with loop_cm:
                outt = outt_pool.tile([128, N_G * 4 * N_ACT], f32)
                for g in range(N_G):
                    arena = arena_pool.tile([128, SG * 512], bf16)
                    ot = None
                    for sl in range(SG):
                        st = g * SG + sl
                        if st % DMA_ST == 0:
                            ot = ot_pool.tile([128, DMA_ST * NT], bf16)
                            eng = nc.scalar if (st // DMA_ST) % 2 else nc.sync
                            eng.dma_start(
                                out=ot[:],
                                in_=O_ap[st * NT:(st + DMA_ST) * NT, :],
                                transpose=True)
                        sub = ot[:, (st % DMA_ST) * NT:(st % DMA_ST + 1) * NT]
                        # stream agent-major so arena lands pair-major and
                        # the decoder reads contiguous 16-col runs
                        otr = sub.rearrange("p (b a) -> p a b", a=N_AGENTS)
                        ph = ph_pool.tile([128, 512], f32)
                        nc.tensor.matmul(ph[0:64, :], wenc[:],
                                         otr[:, 0:32, :],
                                         start=True, stop=True,
                                         tile_position=(0, 0))
                        nc.tensor.matmul(ph[64:128, :], wenc[:],
                                         otr[:, 32:64, :],
                                         start=True, stop=True,
                                         tile_position=(0, 64))
                        nc.scalar.activation(
                            out=arena[:, sl * 512:(sl + 1) * 512],
                            in_=ph[:],
                            func=mybir.ActivationFunctionType.Sigmoid,
                            bias=benc[:])

                    # decoder: accumulate agent pairs into one psum strip
                    av = arena[:].rearrange("q (st a b) -> q st a b",
                                            a=32, b=16)
                    pd = pd_pool.tile([32, GB], f32)
                    for p in range(32):
                        nc.tensor.matmul(pd[:], zpair[:, p, :],
                                         av[:, :, p, :],
                                         start=(p == 0), stop=(p == 31))

                    # + r, transpose to batch-major
                    sab = outsb_pool.tile([32, GB], f32)
                    nc.scalar.add(sab[:], pd[:], add=r2[:])
                    po = po_pool.tile([128, 4 * N_ACT], f32)
                    for ch in range(4):
                        nc.tensor.matmul(
                            po[:, ch * N_ACT:(ch + 1) * N_ACT],
                            sab[:, ch * 128:(ch + 1) * 128], idf[:],
                            start=True, stop=True)
                    nc.vector.tensor_copy(
                        outt[:, g * 4 * N_ACT:(g + 1) * 4 * N_ACT], po[:])

                nc.scalar.dma_start(
                    out=out_ap.rearrange("(g ch p) c -> p g ch c",
                                         g=N_G, ch=4, p=128),
                    in_=outt[:].rearrange("p (g ch c) -> p g ch c",
                                          g=N_G, ch=4))

    nc.compile()
    return nc


def _prep_inputs(inputs):
    W_enc = np.asarray(inputs["W_enc"], dtype=np.float32)
    b_enc = np.asarray(inputs["b_enc"], dtype=np.float32)
    Ws = [np.asarray(inputs[f"W{k}"], dtype=np.float32) for k in (1, 2, 3, 4)]
    bs = [np.asarray(inputs[f"b{k}"], dtype=np.float32) for k in (1, 2, 3, 4)]
    W_dec = np.asarray(inputs["W_dec"], dtype=np.float32)
    b_dec = np.asarray(inputs["b_dec"], dtype=np.float32)

    Zdec, r = _fold_weights(W_enc, b_enc, Ws, bs, W_dec, b_dec)
    zdev = np.ascontiguousarray(Zdec.transpose(1, 0, 2))  # [64 d, 64 a, 32]
    zpair = np.ascontiguousarray(np.concatenate(
        [zdev[:, 0:32, :], zdev[:, 32:64, :]], axis=0)).astype(
            ml_dtypes.bfloat16)                           # [128, 32, 32]
    benc128 = np.concatenate([b_enc, b_enc]).reshape(128, 1).astype(np.float32)
    r2 = r.reshape(32, 1).astype(np.float32)

    O = np.asarray(inputs["O"], dtype=np.float32)
    Obf = O.astype(ml_dtypes.bfloat16)
    common = {
        "Wenc": np.ascontiguousarray(W_enc).astype(ml_dtypes.bfloat16),
        "benc128": benc128,
        "Zpair": zpair,
        "r2": r2,
        "ident32": np.eye(32, dtype=np.float32),
    }
    in_maps = []
    for c in range(N_CORES):
        o_shard = np.ascontiguousarray(
            Obf[c * B_CORE:(c + 1) * B_CORE].reshape(TOK_CORE, OBS_DIM))
        in_maps.append({"Obf": o_shard, **common})
    return in_maps


def _run(inputs, trace=False):
    from concourse.bass_utils import run_bass_kernel_spmd

    if "nc" not in _CACHE:
        _CACHE["nc"] = _build()
    nc = _CACHE["nc"]
    in_maps = _prep_inputs(inputs)
    res = run_bass_kernel_spmd(nc, in_maps, core_ids=list(range(N_CORES)),
                               trace=trace)
    out = np.concatenate(
        [res.results[c]["out"] for c in range(N_CORES)], axis=0)
    return out.astype(np.float32), res


def kernel(**inputs):
    out, _ = _run(inputs, trace=False)
    return out


# revision 11
# speedup vs baseline: 3.4363x; 1.3455x over previous
"""Trainium2 Bass kernel for nn_CommNetActor (gnn_message_passing).

Algebraic collapse: every comm layer is linear (no activation), so the whole
post-sigmoid network folds into per-agent decoder matrices on the host:

    out[b] = sum_a sigmoid(O[b,a] @ W_enc + b_enc) @ Z_a + r

Device pipeline (batch-sharded, 8192/8 = 1024 batches = 65536 tokens/core):
  - host casts O to bf16 (same numerics as the old on-device GPSIMD cast)
  - HWDGE xbar DMA-transpose loads O feature-major straight from HBM
    (no PE transposes, no GPSIMD cast, no PSUM->SBUF copies)
  - encoder: two col-group-tiled bf16 matmuls put agents a<32 on PSUM
    partitions 0-63 and a>=32 on 64-127
  - ACT sigmoid (+bias) -> bf16 arena [128, batch*pair] layout
  - decoder: 32 K=128 matmuls per group (two agents per matmul) accumulate
    over a PSUM strip [32, batches]
  - +r bias, PE transpose back to batch-major, single batched output store
"""

import sys
import numpy as np

sys.path.insert(0, "/opt/trn_rl_repo")

import ml_dtypes

BATCH, N_AGENTS, OBS_DIM, D, N_ACT = 8192, 64, 128, 64, 32
N_CORES = 8
B_CORE = BATCH // N_CORES              # 1024 batches per core
TOK_CORE = B_CORE * N_AGENTS           # 65536 tokens per core
NT = 1024                              # tokens per super-tile (16 batches)
N_ST = TOK_CORE // NT                  # 64 super-tiles
SG = 32                                # super-tiles per decoder group
N_G = N_ST // SG                       # 2 groups
GB = SG * (NT // N_AGENTS)             # 512 batches per group
DMA_ST = 4                             # super-tiles per input DMA (1 MB)

_CACHE = {}


def _fold_weights(W_enc, b_enc, Ws, bs, W_dec, b_dec):
    """Fold the 4 linear comm layers + decoder into Zdec [64,64,32] and r[32]."""
    A = N_AGENTS
    I = np.eye(D)
    Gamma = I.copy()
    E = np.zeros((D, D))
    c = np.zeros(D)
    Lam = I.copy()
    d = np.zeros(D)
    for W, b in zip(Ws, bs):
        W = W.astype(np.float64)
        b = b.astype(np.float64)
        Wt, Wb = W[:D], W[D:]
        V = Wb / A
        Wp = Wt - V
        U = Wt + (A - 1) * V
        E, c = E @ Wp + Lam @ V, c @ Wp + d @ V + b
        Gamma = Gamma @ Wp
        Lam, d = Lam @ U, d @ U + A * b
    Wd = W_dec.astype(np.float64).reshape(A, D, N_ACT)
    Wsum = Wd.sum(axis=0)
    Zdec = np.einsum("ij,ajk->aik", Gamma, Wd) + (E @ Wsum)[None]
    r = c @ Wsum + b_dec.astype(np.float64)
    return Zdec, r


def _build(loop_reps=1):
    import concourse.bass as bass
    import concourse.bacc as bacc
    import concourse.tile as tile
    from concourse import mybir
    from concourse._compat import get_trn_type

    f32 = mybir.dt.float32
    bf16 = mybir.dt.bfloat16

    nc = bacc.Bacc(get_trn_type() or "TRN2", target_bir_lowering=False,
                   debug=False, enable_asserts=True, num_devices=N_CORES)

    O_d = nc.dram_tensor("Obf", [TOK_CORE, OBS_DIM], bf16, kind="ExternalInput")
    wenc_d = nc.dram_tensor("Wenc", [OBS_DIM, D], bf16, kind="ExternalInput")
    benc_d = nc.dram_tensor("benc128", [128, 1], f32, kind="ExternalInput")
    zpair_d = nc.dram_tensor("Zpair", [128, 32, N_ACT], bf16,
                             kind="ExternalInput")
    r_d = nc.dram_tensor("r2", [32, 1], f32, kind="ExternalInput")
    idf_d = nc.dram_tensor("ident32", [32, 32], f32, kind="ExternalInput")
    out_d = nc.dram_tensor("out", [B_CORE, N_ACT], f32, kind="ExternalOutput")

    O_ap = O_d.ap()
    out_ap = out_d.ap()

    with tile.TileContext(nc) as tc:
        with (
            tc.tile_pool(name="const", bufs=1) as const_pool,
            tc.tile_pool(name="otsb", bufs=4) as ot_pool,
            tc.tile_pool(name="arena", bufs=2) as arena_pool,
            tc.tile_pool(name="outsb", bufs=2) as outsb_pool,
            tc.tile_pool(name="outt", bufs=2) as outt_pool,
            tc.tile_pool(name="ph", bufs=4, space="PSUM") as ph_pool,
            tc.tile_pool(name="pd", bufs=2, space="PSUM") as pd_pool,
            tc.tile_pool(name="po", bufs=1, space="PSUM") as po_pool,
        ):
            # constants
            wenc = const_pool.tile([OBS_DIM, D], bf16)
            nc.sync.dma_start(out=wenc[:], in_=wenc_d.ap())
            benc = const_pool.tile([128, 1], f32)
            nc.sync.dma_start(out=benc[:], in_=benc_d.ap())
            zpair = const_pool.tile([128, 32, N_ACT], bf16)
            nc.sync.dma_start(out=zpair[:], in_=zpair_d.ap())
            r2 = const_pool.tile([32, 1], f32)
            nc.sync.dma_start(out=r2[:], in_=r_d.ap())
            idf = const_pool.tile([32, 32], f32)
            nc.sync.dma_start(out=idf[:], in_=idf_d.ap())

            import contextlib
            loop_cm = (tc.For_i(0, loop_reps, 1) if loop_reps > 1
                       else contextlib.nullcontext())
            with loop_cm:
                outt = outt_pool.tile([128, N_G * 4 * N_ACT], f32)
                for g in range(N_G):
                    arena = arena_pool.tile([128, SG * 512], bf16)
                    ot = None
                    for sl in range(SG):
                        st = g * SG + sl
                        if st % DMA_ST == 0:
                            ot = ot_pool.tile([128, DMA_ST * NT], bf16)
                            nc.sync.dma_start(
                                out=ot[:],
                                in_=O_ap[st * NT:(st + DMA_ST) * NT, :],
                                transpose=True)
                        sub = ot[:, (st % DMA_ST) * NT:(st % DMA_ST + 1) * NT]
                        # stream agent-major so arena lands pair-major and
                        # the decoder reads contiguous 16-col runs
                        otr = sub.rearrange("p (b a) -> p a b", a=N_AGENTS)
                        ph = ph_pool.tile([128, 512], f32)
                        nc.tensor.matmul(ph[0:64, :], wenc[:],
                                         otr[:, 0:32, :],
                                         start=True, stop=True,
                                         tile_position=(0, 0))
                        nc.tensor.matmul(ph[64:128, :], wenc[:],
                                         otr[:, 32:64, :],
                                         start=True, stop=True,
                                         tile_position=(0, 64))
                        nc.scalar.activation(
                            out=arena[:, sl * 512:(sl + 1) * 512],
                            in_=ph[:],
                            func=mybir.ActivationFunctionType.Sigmoid,
                            bias=benc[:])

                    # decoder: accumulate agent pairs into one psum strip
                    av = arena[:].rearrange("q (st a b) -> q st a b",
                                            a=32, b=16)
                    pd = pd_pool.tile([32, GB], f32)
                    for p in range(32):
                        nc.tensor.matmul(pd[:], zpair[:, p, :],
                                         av[:, :, p, :],
                                         start=(p == 0), stop=(p == 31))

                    # + r, transpose to batch-major
                    sab = outsb_pool.tile([32, GB], f32)
                    nc.scalar.add(sab[:], pd[:], add=r2[:])
                    po = po_pool.tile([128, 4 * N_ACT], f32)
                    for ch in range(4):
                        nc.tensor.matmul(
                            po[:, ch * N_ACT:(ch + 1) * N_ACT],
                            sab[:, ch * 128:(ch + 1) * 128], idf[:],
                            start=True, stop=True)
                    nc.vector.tensor_copy(
                        outt[:, g * 4 * N_ACT:(g + 1) * 4 * N_ACT], po[:])

                nc.sync.dma_start(
                    out=out_ap.rearrange("(g ch p) c -> p g ch c",
                                         g=N_G, ch=4, p=128),
                    in_=outt[:].rearrange("p (g ch c) -> p g ch c",
                                          g=N_G, ch=4))

    nc.compile()
    return nc


def _prep_inputs(inputs):
    W_enc = np.asarray(inputs["W_enc"], dtype=np.float32)
    b_enc = np.asarray(inputs["b_enc"], dtype=np.float32)
    Ws = [np.asarray(inputs[f"W{k}"], dtype=np.float32) for k in (1, 2, 3, 4)]
    bs = [np.asarray(inputs[f"b{k}"], dtype=np.float32) for k in (1, 2, 3, 4)]
    W_dec = np.asarray(inputs["W_dec"], dtype=np.float32)
    b_dec = np.asarray(inputs["b_dec"], dtype=np.float32)

    Zdec, r = _fold_weights(W_enc, b_enc, Ws, bs, W_dec, b_dec)
    zdev = np.ascontiguousarray(Zdec.transpose(1, 0, 2))  # [64 d, 64 a, 32]
    zpair = np.ascontiguousarray(np.concatenate(
        [zdev[:, 0:32, :], zdev[:, 32:64, :]], axis=0)).astype(
            ml_dtypes.bfloat16)                           # [128, 32, 32]
    benc128 = np.concatenate([b_enc, b_enc]).reshape(128, 1).astype(np.float32)
    r2 = r.reshape(32, 1).astype(np.float32)

    O = np.asarray(inputs["O"], dtype=np.float32)
    Obf = O.astype(ml_dtypes.bfloat16)
    common = {
        "Wenc": np.ascontiguousarray(W_enc).astype(ml_dtypes.bfloat16),
        "benc128": benc128,
        "Zpair": zpair,
        "r2": r2,
        "ident32": np.eye(32, dtype=np.float32),
    }
    in_maps = []
    for c in range(N_CORES):
        o_shard = np.ascontiguousarray(
            Obf[c * B_CORE:(c + 1) * B_CORE].reshape(TOK_CORE, OBS_DIM))
        in_maps.append({"Obf": o_shard, **common})
    return in_maps


def _run(inputs, trace=False):
    from concourse.bass_utils import run_bass_kernel_spmd

    if "nc" not in _CACHE:
        _CACHE["nc"] = _build()
    nc = _CACHE["nc"]
    in_maps = _prep_inputs(inputs)
    res = run_bass_kernel_spmd(nc, in_maps, core_ids=list(range(N_CORES)),
                               trace=trace)
    out = np.concatenate(
        [res.results[c]["out"] for c in range(N_CORES)], axis=0)
    return out.astype(np.float32), res


def kernel(**inputs):
    out, _ = _run(inputs, trace=False)
    return out


# revision 15
# speedup vs baseline: 3.4455x; 1.0027x over previous
"""Trainium2 Bass kernel for nn_CommNetActor (gnn_message_passing).

Algebraic collapse: every comm layer is linear (no activation), so the whole
post-sigmoid network folds into per-agent decoder matrices on the host:

    out[b] = sum_a sigmoid(O[b,a] @ W_enc + b_enc) @ Z_a + r

Device pipeline (batch-sharded, 8192/8 = 1024 batches = 65536 tokens/core):
  - host casts O to bf16 (same numerics as the old on-device GPSIMD cast)
  - HWDGE xbar DMA-transpose loads O feature-major straight from HBM
    (no PE transposes, no GPSIMD cast, no PSUM->SBUF copies)
  - encoder: two col-group-tiled bf16 matmuls put agents a<32 on PSUM
    partitions 0-63 and a>=32 on 64-127
  - ACT sigmoid (+bias) -> bf16 arena [128, batch*pair] layout
  - decoder: 32 K=128 matmuls per group (two agents per matmul) accumulate
    over a PSUM strip [32, batches]
  - +r bias, PE transpose back to batch-major, single batched output store
"""

import sys
import numpy as np

sys.path.insert(0, "/opt/trn_rl_repo")

import ml_dtypes

BATCH, N_AGENTS, OBS_DIM, D, N_ACT = 8192, 64, 128, 64, 32
N_CORES = 8
B_CORE = BATCH // N_CORES              # 1024 batches per core
TOK_CORE = B_CORE * N_AGENTS           # 65536 tokens per core
NT = 1024                              # tokens per super-tile (16 batches)
N_ST = TOK_CORE // NT                  # 64 super-tiles
SG = 32                                # super-tiles per decoder group
N_G = N_ST // SG                       # 2 groups
GB = SG * (NT // N_AGENTS)             # 512 batches per group
DMA_ST = 8                             # super-tiles per input DMA (2 MB)

_CACHE = {}


def _fold_weights(W_enc, b_enc, Ws, bs, W_dec, b_dec):
    """Fold the 4 linear comm layers + decoder into Zdec [64,64,32] and r[32]."""
    A = N_AGENTS
    I = np.eye(D)
    Gamma = I.copy()
    E = np.zeros((D, D))
    c = np.zeros(D)
    Lam = I.copy()
    d = np.zeros(D)
    for W, b in zip(Ws, bs):
        W = W.astype(np.float64)
        b = b.astype(np.float64)
        Wt, Wb = W[:D], W[D:]
        V = Wb / A
        Wp = Wt - V
        U = Wt + (A - 1) * V
        E, c = E @ Wp + Lam @ V, c @ Wp + d @ V + b
        Gamma = Gamma @ Wp
        Lam, d = Lam @ U, d @ U + A * b
    Wd = W_dec.astype(np.float64).reshape(A, D, N_ACT)
    Wsum = Wd.sum(axis=0)
    Zdec = np.einsum("ij,ajk->aik", Gamma, Wd) + (E @ Wsum)[None]
    r = c @ Wsum + b_dec.astype(np.float64)
    return Zdec, r


def _build(loop_reps=1):
    import concourse.bass as bass
    import concourse.bacc as bacc
    import concourse.tile as tile
    from concourse import mybir
    from concourse._compat import get_trn_type

    f32 = mybir.dt.float32
    bf16 = mybir.dt.bfloat16

    nc = bacc.Bacc(get_trn_type() or "TRN2", target_bir_lowering=False,
                   debug=False, enable_asserts=True, num_devices=N_CORES)

    O_d = nc.dram_tensor("Obf", [TOK_CORE, OBS_DIM], bf16, kind="ExternalInput")
    wenc_d = nc.dram_tensor("Wenc", [OBS_DIM, D], bf16, kind="ExternalInput")
    benc_d = nc.dram_tensor("benc128", [128, 1], f32, kind="ExternalInput")
    zpair_d = nc.dram_tensor("Zpair", [128, 32, N_ACT], bf16,
                             kind="ExternalInput")
    r_d = nc.dram_tensor("r2", [32, 1], f32, kind="ExternalInput")
    idf_d = nc.dram_tensor("ident32", [32, 32], f32, kind="ExternalInput")
    out_d = nc.dram_tensor("out", [B_CORE, N_ACT], f32, kind="ExternalOutput")

    O_ap = O_d.ap()
    out_ap = out_d.ap()

    with tile.TileContext(nc) as tc:
        with (
            tc.tile_pool(name="const", bufs=1) as const_pool,
            tc.tile_pool(name="otsb", bufs=3) as ot_pool,
            tc.tile_pool(name="arena", bufs=2) as arena_pool,
            tc.tile_pool(name="outsb", bufs=2) as outsb_pool,
            tc.tile_pool(name="outt", bufs=2) as outt_pool,
            tc.tile_pool(name="ph", bufs=4, space="PSUM") as ph_pool,
            tc.tile_pool(name="pd", bufs=2, space="PSUM") as pd_pool,
            tc.tile_pool(name="po", bufs=1, space="PSUM") as po_pool,
        ):
            # constants
            wenc = const_pool.tile([OBS_DIM, D], bf16)
            nc.sync.dma_start(out=wenc[:], in_=wenc_d.ap())
            benc = const_pool.tile([128, 1], f32)
            nc.sync.dma_start(out=benc[:], in_=benc_d.ap())
            zpair = const_pool.tile([128, 32, N_ACT], bf16)
            nc.sync.dma_start(out=zpair[:], in_=zpair_d.ap())
            r2 = const_pool.tile([32, 1], f32)
            nc.sync.dma_start(out=r2[:], in_=r_d.ap())
            idf = const_pool.tile([32, 32], f32)
            nc.sync.dma_start(out=idf[:], in_=idf_d.ap())

            import contextlib
            loop_cm = (tc.For_i(0, loop_reps, 1) if loop_reps > 1
                       else contextlib.nullcontext())
            with loop_cm:
                outt = outt_pool.tile([128, N_G * 4 * N_ACT], f32)
                for g in range(N_G):
                    arena = arena_pool.tile([128, SG * 512], bf16)
                    ot = None
                    for sl in range(SG):
                        st = g * SG + sl
                        if st % DMA_ST == 0:
                            ot = ot_pool.tile([128, DMA_ST * NT], bf16)
                            nc.sync.dma_start(
                                out=ot[:],
                                in_=O_ap[st * NT:(st + DMA_ST) * NT, :],
                                transpose=True)
                        sub = ot[:, (st % DMA_ST) * NT:(st % DMA_ST + 1) * NT]
                        # stream agent-major so arena lands pair-major and
                        # the decoder reads contiguous 16-col runs
                        otr = sub.rearrange("p (b a) -> p a b", a=N_AGENTS)
                        ph = ph_pool.tile([128, 512], f32)
                        nc.tensor.matmul(ph[0:64, :], wenc[:],
                                         otr[:, 0:32, :],
                                         start=True, stop=True,
                                         tile_position=(0, 0))
                        nc.tensor.matmul(ph[64:128, :], wenc[:],
                                         otr[:, 32:64, :],
                                         start=True, stop=True,
                                         tile_position=(0, 64))
                        # scatter into pair-major slabs: arena col layout is
                        # (pair a: 32) x (st: SG) x (batch b: 16), so the
                        # decoder's moving operand is fully contiguous
                        a2 = arena[:].rearrange("q (a st b) -> q a st b",
                                                a=32, st=SG)
                        nc.scalar.activation(
                            out=a2[:, :, sl, :],
                            in_=ph[:],
                            func=mybir.ActivationFunctionType.Sigmoid,
                            bias=benc[:])

                    # decoder: accumulate agent pairs into one psum strip;
                    # each pair's operand is one contiguous 512-col slab
                    pd = pd_pool.tile([32, GB], f32)
                    for p in range(32):
                        nc.tensor.matmul(pd[:], zpair[:, p, :],
                                         arena[:, p * 512:(p + 1) * 512],
                                         start=(p == 0), stop=(p == 31))

                    # + r, transpose to batch-major
                    sab = outsb_pool.tile([32, GB], f32)
                    nc.scalar.add(sab[:], pd[:], add=r2[:])
                    po = po_pool.tile([128, 4 * N_ACT], f32)
                    for ch in range(4):
                        nc.tensor.matmul(
                            po[:, ch * N_ACT:(ch + 1) * N_ACT],
                            sab[:, ch * 128:(ch + 1) * 128], idf[:],
                            start=True, stop=True)
                    nc.vector.tensor_copy(
                        outt[:, g * 4 * N_ACT:(g + 1) * 4 * N_ACT], po[:])

                nc.sync.dma_start(
                    out=out_ap.rearrange("(g ch p) c -> p g ch c",
                                         g=N_G, ch=4, p=128),
                    in_=outt[:].rearrange("p (g ch c) -> p g ch c",
                                          g=N_G, ch=4))

    nc.compile()
    return nc


def _prep_inputs(inputs):
    W_enc = np.asarray(inputs["W_enc"], dtype=np.float32)
    b_enc = np.asarray(inputs["b_enc"], dtype=np.float32)
    Ws = [np.asarray(inputs[f"W{k}"], dtype=np.float32) for k in (1, 2, 3, 4)]
    bs = [np.asarray(inputs[f"b{k}"], dtype=np.float32) for k in (1, 2, 3, 4)]
    W_dec = np.asarray(inputs["W_dec"], dtype=np.float32)
    b_dec = np.asarray(inputs["b_dec"], dtype=np.float32)

    Zdec, r = _fold_weights(W_enc, b_enc, Ws, bs, W_dec, b_dec)
    zdev = np.ascontiguousarray(Zdec.transpose(1, 0, 2))  # [64 d, 64 a, 32]
    zpair = np.ascontiguousarray(np.concatenate(
        [zdev[:, 0:32, :], zdev[:, 32:64, :]], axis=0)).astype(
            ml_dtypes.bfloat16)                           # [128, 32, 32]
    benc128 = np.concatenate([b_enc, b_enc]).reshape(128, 1).astype(np.float32)
    r2 = r.reshape(32, 1).astype(np.float32)

    O = np.asarray(inputs["O"], dtype=np.float32)
    Obf = O.astype(ml_dtypes.bfloat16)
    common = {
        "Wenc": np.ascontiguousarray(W_enc).astype(ml_dtypes.bfloat16),
        "benc128": benc128,
        "Zpair": zpair,
        "r2": r2,
        "ident32": np.eye(32, dtype=np.float32),
    }
    in_maps = []
    for c in range(N_CORES):
        o_shard = np.ascontiguousarray(
            Obf[c * B_CORE:(c + 1) * B_CORE].reshape(TOK_CORE, OBS_DIM))
        in_maps.append({"Obf": o_shard, **common})
    return in_maps


def _run(inputs, trace=False):
    from concourse.bass_utils import run_bass_kernel_spmd

    if "nc" not in _CACHE:
        _CACHE["nc"] = _build()
    nc = _CACHE["nc"]
    in_maps = _prep_inputs(inputs)
    res = run_bass_kernel_spmd(nc, in_maps, core_ids=list(range(N_CORES)),
                               trace=trace)
    out = np.concatenate(
        [res.results[c]["out"] for c in range(N_CORES)], axis=0)
    return out.astype(np.float32), res


def kernel(**inputs):
    out, _ = _run(inputs, trace=False)
    return out
